# revision 6
# baseline (speedup 1.0000x reference)
"""Causal BoW (running mean over T) Trainium2 kernel.

out[b, t, c] = sum_{s<=t} x[b, s, c] / (t+1)   for x of shape [32, 2048, 512] f32.

Sharding: batch B=32 across 8 NeuronCores (4 samples each), no cross-core comms.

Per-core algorithm (per sample [T=2048, C=512]):
  - SBUF layout [p=128, (j, c)] with t = j*128 + p, j = block index (16 blocks).
  - Block scan: psum_j = U128^T.T @ x_j  (U128 upper-triangular ones -> causal
    prefix sum within the 128-row block), on the tensor engine.
  - Block offsets: 16 accumulating matmuls with a "step" selector weight
    (step[p, m] = 1 if m > k for block k) produce off[m, c] = sum_{k<m} tot_k
    directly in one PSUM bank; evicted to SBUF.
  - Offset broadcast: psum_j += rowsel_j^T.T @ off  (K=16 matmul picking row j,
    broadcast across all 128 output partitions).
  - Eviction: scalar-engine activation Copy with per-partition scale
    recip[p, j] = 1/(j*128+p+1) applied while moving PSUM -> SBUF.
"""

import numpy as np

import concourse.bass as bass
import concourse.bacc as bacc
import concourse.mybir as mybir
from concourse import tile
from concourse.bass_utils import run_bass_kernel_spmd

B, T, C = 32, 2048, 512
N_CORES = 8
BS = B // N_CORES          # samples per core
P = 128                    # partitions / T-block size
NBLK = T // P              # 16 blocks per sample
F32 = mybir.dt.float32

_cache = {}


def _build():
    nc = bacc.Bacc()
    x = nc.dram_tensor("x", [BS, T, C], F32, kind="ExternalInput")
    u128 = nc.dram_tensor("u128", [P, P], F32, kind="ExternalInput")
    stepm = nc.dram_tensor("stepm", [P, NBLK * NBLK], F32, kind="ExternalInput")
    rowsel = nc.dram_tensor("rowsel", [NBLK, NBLK * P], F32, kind="ExternalInput")
    recip = nc.dram_tensor("recip", [P, NBLK], F32, kind="ExternalInput")
    y = nc.dram_tensor("y", [BS, T, C], F32, kind="ExternalOutput")

    HALF = NBLK // 2 * C   # free-size of half a sample in SBUF

    with tile.TileContext(nc) as tc:
        with (
            tc.tile_pool(name="singles", bufs=1) as singles,
            tc.tile_pool(name="xp", bufs=4) as xpool,
            tc.tile_pool(name="op", bufs=4) as opool,
            tc.tile_pool(name="offp", bufs=2) as offpool,
            tc.tile_pool(name="pblk", bufs=5, space="PSUM") as pblk,
            tc.tile_pool(name="poff", bufs=2, space="PSUM") as poff,
        ):
            u_t = singles.tile([P, P], F32)
            nc.sync.dma_start(out=u_t[:], in_=u128[:])
            step_t = singles.tile([P, NBLK * NBLK], F32)
            nc.sync.dma_start(out=step_t[:], in_=stepm[:])
            rowsel_t = singles.tile([NBLK, NBLK * P], F32)
            nc.sync.dma_start(out=rowsel_t[:], in_=rowsel[:])
            recip_t = singles.tile([P, NBLK], F32)
            nc.sync.dma_start(out=recip_t[:], in_=recip[:])

            for b in range(BS):
                xs = x[b].rearrange("(j p) c -> p j c", p=P)   # [128, 16, 512]
                ys = y[b].rearrange("(j p) c -> p j c", p=P)

                halves = []
                for h in range(2):
                    xt = xpool.tile([P, HALF], F32, tag="xt")
                    xt3 = xt.rearrange("p (j c) -> p j c", c=C)
                    nc.sync.dma_start(
                        out=xt3[:], in_=xs[:, h * (NBLK // 2):(h + 1) * (NBLK // 2), :]
                    )
                    halves.append(xt)

                # off[m, c] = sum_{k < m} (block-k column sum), in one PSUM bank
                offp_t = poff.tile([NBLK, C], F32)
                for k in range(NBLK):
                    nc.tensor.matmul(
                        offp_t[:],
                        step_t[:, k * NBLK:(k + 1) * NBLK],
                        halves[k // 8][:, (k % 8) * C:(k % 8 + 1) * C],
                        start=(k == 0),
                        stop=(k == NBLK - 1),
                    )
                off_sb = offpool.tile([NBLK, C], F32)
                nc.vector.tensor_copy(out=off_sb[:], in_=offp_t[:])

                for h in range(2):
                    ot = opool.tile([P, HALF], F32, tag="ot")
                    for jj in range(NBLK // 2):
                        j = h * (NBLK // 2) + jj
                        pb = pblk.tile([P, C], F32)
                        nc.tensor.matmul(
                            pb[:],
                            u_t[:],
                            halves[h][:, jj * C:(jj + 1) * C],
                            start=True,
                            stop=(j == 0),
                        )
                        if j > 0:
                            nc.tensor.matmul(
                                pb[:],
                                rowsel_t[:, j * P:(j + 1) * P],
                                off_sb[:],
                                start=False,
                                stop=True,
                            )
                        nc.scalar.mul(ot[:, jj * C:(jj + 1) * C], pb[:],
                                      recip_t[:, j:j + 1])
                    ot3 = ot.rearrange("p (j c) -> p j c", c=C)
                    nc.sync.dma_start(
                        out=ys[:, h * (NBLK // 2):(h + 1) * (NBLK // 2), :], in_=ot3[:]
                    )
    nc.finalize()
    return nc


def _consts():
    u = np.triu(np.ones((P, P), dtype=np.float32))
    step = np.zeros((P, NBLK * NBLK), dtype=np.float32)
    for k in range(NBLK):
        for m in range(NBLK):
            if m > k:
                step[:, k * NBLK + m] = 1.0
    rowsel = np.zeros((NBLK, NBLK * P), dtype=np.float32)
    for j in range(NBLK):
        rowsel[j, j * P:(j + 1) * P] = 1.0
    recip = (1.0 / np.arange(1, T + 1, dtype=np.float32)).reshape(NBLK, P).T.copy()
    return u, step, rowsel, recip


def run(x, trace=False):
    x = np.ascontiguousarray(np.asarray(x, dtype=np.float32))
    assert x.shape == (B, T, C), x.shape
    if "nc" not in _cache:
        _cache["nc"] = _build()
    nc = _cache["nc"]
    u, step, rowsel, recip = _consts()
    in_maps = [
        {
            "x": np.ascontiguousarray(x[i * BS:(i + 1) * BS]),
            "u128": u,
            "stepm": step,
            "rowsel": rowsel,
            "recip": recip,
        }
        for i in range(N_CORES)
    ]
    res = run_bass_kernel_spmd(nc, in_maps, list(range(N_CORES)), trace=trace)
    y = np.concatenate([res.results[i]["y"] for i in range(N_CORES)], axis=0)
    return y, res.exec_time_ns


def kernel(x):
    y, _ = run(x, trace=False)
    return y


# revision 10
# speedup vs baseline: 1.4091x; 1.4091x over previous
"""Causal BoW (running mean over T) Trainium2 kernel.

out[b, t, c] = sum_{s<=t} x[b, s, c] / (t+1)   for x of shape [32, 2048, 512] f32.

Sharding: batch B=32 across 8 NeuronCores (4 samples each), no cross-core comms.

Per-core algorithm (per sample [T=2048, C=512]):
  - SBUF layout [p=128, (j, c)] with t = j*128 + p, j = block index (16 blocks).
  - f32 matmuls cost 4 cycles/row on the PE; float32r costs 1 cycle/row but
    keeps only 11 mantissa bits. So x is split on-chip into
    x_hi = round_f32r(x) (ACT copy) and x_lo = round_f32r(x - x_hi) (DVE sub);
    streaming both through the PE reconstructs full fp32 precision (verified
    bit-exact on HW) at 2 cycles/row total.
  - Block scan: psum_j = U128^T.T @ x_hi_j + U128^T.T @ x_lo_j (U128 =
    upper-triangular ones -> causal prefix sum within the 128-row block).
  - Block offsets: accumulating matmuls with a "step" selector weight
    (step[p, m] = 1 if m > k for block k) produce off[m, c] = sum_{k<m} tot_k
    directly in one PSUM bank; split to off_hi/off_lo f32r tiles.
  - Offset broadcast: psum_j += rowsel_j^T.T @ off_hi + rowsel_j^T.T @ off_lo
    (K=16 matmuls picking row j, broadcast across all 128 output partitions).
  - Eviction: Copy with per-partition scale recip[p, j] = 1/(j*128+p+1)
    applied while moving PSUM -> SBUF, alternating ACT/DVE.
"""

import numpy as np

import concourse.bass as bass
import concourse.bacc as bacc
import concourse.mybir as mybir
from concourse import tile
from concourse.bass_utils import run_bass_kernel_spmd

B, T, C = 32, 2048, 512
N_CORES = 8
BS = B // N_CORES          # samples per core
P = 128                    # partitions / T-block size
NBLK = T // P              # 16 blocks per sample
F32 = mybir.dt.float32
F32R = mybir.dt.float32r

_cache = {}


def _build():
    nc = bacc.Bacc()
    x = nc.dram_tensor("x", [BS, T, C], F32, kind="ExternalInput")
    u128 = nc.dram_tensor("u128", [P, P], F32R, kind="ExternalInput")
    stepm = nc.dram_tensor("stepm", [P, NBLK * NBLK], F32R, kind="ExternalInput")
    rowsel = nc.dram_tensor("rowsel", [NBLK, NBLK * P], F32R, kind="ExternalInput")
    recip = nc.dram_tensor("recip", [P, NBLK], F32, kind="ExternalInput")
    y = nc.dram_tensor("y", [BS, T, C], F32, kind="ExternalOutput")

    NH = NBLK // 2            # blocks per half-sample
    HALF = NH * C             # free-size of half a sample in SBUF

    with tile.TileContext(nc) as tc:
        with (
            tc.tile_pool(name="singles", bufs=1) as singles,
            tc.tile_pool(name="xp", bufs=2) as xpool,
            tc.tile_pool(name="xhp", bufs=2) as xhpool,
            tc.tile_pool(name="xlp", bufs=2) as xlpool,
            tc.tile_pool(name="op", bufs=3) as opool,
            tc.tile_pool(name="offp", bufs=2) as offpool,
            tc.tile_pool(name="pblk", bufs=5, space="PSUM") as pblk,
            tc.tile_pool(name="poff", bufs=2, space="PSUM") as poff,
        ):
            u_t = singles.tile([P, P], F32R)
            nc.sync.dma_start(out=u_t[:], in_=u128[:])
            step_t = singles.tile([P, NBLK * NBLK], F32R)
            nc.sync.dma_start(out=step_t[:], in_=stepm[:])
            rowsel_t = singles.tile([NBLK, NBLK * P], F32R)
            nc.sync.dma_start(out=rowsel_t[:], in_=rowsel[:])
            recip_t = singles.tile([P, NBLK], F32)
            nc.sync.dma_start(out=recip_t[:], in_=recip[:])

            for b in range(BS):
                xs = x[b].rearrange("(j p) c -> p j c", p=P)   # [128, 16, 512]
                ys = y[b].rearrange("(j p) c -> p j c", p=P)

                his, los = [], []
                for h in range(2):
                    xt = xpool.tile([P, HALF], F32, tag="xt")
                    xt3 = xt.rearrange("p (j c) -> p j c", c=C)
                    nc.sync.dma_start(out=xt3[:], in_=xs[:, h * NH:(h + 1) * NH, :])
                    xh = xhpool.tile([P, HALF], F32R, tag="xh")
                    nc.scalar.copy(out=xh[:], in_=xt[:])
                    xl = xlpool.tile([P, HALF], F32R, tag="xl")
                    nc.vector.tensor_sub(out=xl[:], in0=xt[:], in1=xh[:].bitcast(F32))
                    his.append(xh)
                    los.append(xl)

                # off[m, c] = sum_{k < m} (block-k column sum), in one PSUM bank
                offp_t = poff.tile([NBLK, C], F32)
                for k in range(NBLK):
                    sel = step_t[:, k * NBLK:(k + 1) * NBLK]
                    for part, src in ((0, his), (1, los)):
                        nc.tensor.matmul(
                            offp_t[:],
                            sel,
                            src[k // NH][:, (k % NH) * C:(k % NH + 1) * C],
                            start=(k == 0 and part == 0),
                            stop=(k == NBLK - 1 and part == 1),
                        )
                off_hi = offpool.tile([NBLK, C], F32R, tag="offhi")
                nc.scalar.copy(out=off_hi[:], in_=offp_t[:])
                off_lo = offpool.tile([NBLK, C], F32R, tag="offlo")
                nc.vector.tensor_sub(
                    out=off_lo[:], in0=offp_t[:], in1=off_hi[:].bitcast(F32)
                )

                for h in range(2):
                    ot = opool.tile([P, HALF], F32, tag="ot")
                    for jj in range(NH):
                        j = h * NH + jj
                        pb = pblk.tile([P, C], F32)
                        cs = slice(jj * C, (jj + 1) * C)
                        nc.tensor.matmul(pb[:], u_t[:], his[h][:, cs],
                                         start=True, stop=False)
                        nc.tensor.matmul(pb[:], u_t[:], los[h][:, cs],
                                         start=False, stop=(j == 0))
                        if j > 0:
                            rs = rowsel_t[:, j * P:(j + 1) * P]
                            nc.tensor.matmul(pb[:], rs, off_hi[:],
                                             start=False, stop=False)
                            nc.tensor.matmul(pb[:], rs, off_lo[:],
                                             start=False, stop=True)
                        if j % 2 == 0:
                            nc.scalar.mul(ot[:, cs], pb[:], recip_t[:, j:j + 1])
                        else:
                            nc.vector.tensor_scalar_mul(
                                ot[:, cs], pb[:], recip_t[:, j:j + 1]
                            )
                    ot3 = ot.rearrange("p (j c) -> p j c", c=C)
                    nc.sync.dma_start(
                        out=ys[:, h * NH:(h + 1) * NH, :], in_=ot3[:]
                    )
    nc.finalize()
    return nc


def _consts():
    u = np.triu(np.ones((P, P), dtype=np.float32))
    step = np.zeros((P, NBLK * NBLK), dtype=np.float32)
    for k in range(NBLK):
        for m in range(NBLK):
            if m > k:
                step[:, k * NBLK + m] = 1.0
    rowsel = np.zeros((NBLK, NBLK * P), dtype=np.float32)
    for j in range(NBLK):
        rowsel[j, j * P:(j + 1) * P] = 1.0
    recip = (1.0 / np.arange(1, T + 1, dtype=np.float32)).reshape(NBLK, P).T.copy()
    return u, step, rowsel, recip


def run(x, trace=False):
    x = np.ascontiguousarray(np.asarray(x, dtype=np.float32))
    assert x.shape == (B, T, C), x.shape
    if "nc" not in _cache:
        _cache["nc"] = _build()
    nc = _cache["nc"]
    u, step, rowsel, recip = _consts()
    in_maps = [
        {
            "x": np.ascontiguousarray(x[i * BS:(i + 1) * BS]),
            "u128": u,
            "stepm": step,
            "rowsel": rowsel,
            "recip": recip,
        }
        for i in range(N_CORES)
    ]
    res = run_bass_kernel_spmd(nc, in_maps, list(range(N_CORES)), trace=trace)
    y = np.concatenate([res.results[i]["y"] for i in range(N_CORES)], axis=0)
    return y, res.exec_time_ns


def kernel(x):
    y, _ = run(x, trace=False)
    return y
